# revision 5
# baseline (speedup 1.0000x reference)
"""Trainium2 Bass kernel: strided 3x3 conv (stride 2, pad 1) + bias
+ hardswish + mish, data-parallel over batch across 8 NeuronCores.

Problem shapes (hardcoded):
  x:      (16, 64, 256, 256) f32
  weight: (128, 64, 3, 3)    f32
  bias:   (128,)             f32
  out:    (16, 128, 128, 128) f32

Per-core: 2 images. Conv is lowered to 9 tap matmuls (fp32r) per PSUM
bank; the two images are packed into PE row-groups (partitions 0-63 /
64-127, tile_position (0,0)/(64,0)) so their matmuls run concurrently.

mish(h) = h * tanh(softplus(h)) = h * (w-1)/(w+1), w = (1+exp(h))^2,
which needs only {exp, square, identity} — one ACT table set — plus a
fast DVE reciprocal.
"""
import numpy as np

import concourse.bass as bass
import concourse.mybir as mybir
import concourse.tile as tile
from concourse import bacc
from concourse.bass_utils import run_bass_kernel_spmd

F32 = mybir.dt.float32
F32R = mybir.dt.float32r
AFT = mybir.ActivationFunctionType
ALU = mybir.AluOpType

B, CIN, H, W = 16, 64, 256, 256
COUT, KS = 128, 3
HO, WO = 128, 128
NCORE = 8
PER = B // NCORE          # images per core
R = 8                     # output rows per chunk
NCHUNK = HO // R          # 16
RIN = 2 * R + 1           # input row slots per chunk (17)
WP = W + 1                # left-padded width (257)

_CACHE: dict = {}


def _build():
    nc = bacc.Bacc(None, target_bir_lowering=False)
    x_ext = nc.declare_dram_parameter("x", [PER, CIN, H, W], F32, isOutput=False)
    wt_ext = nc.declare_dram_parameter("wt", [128, 9 * COUT], F32, isOutput=False)
    b_ext = nc.declare_dram_parameter("b0", [COUT, 1], F32, isOutput=False)
    out_ext = nc.declare_dram_parameter(
        "out", [PER, COUT, HO, WO], F32, isOutput=True
    )

    with tile.TileContext(nc) as tc:
        with (
            tc.tile_pool(name="const", bufs=1) as cpool,
            tc.tile_pool(name="xin", bufs=3) as xpool,
            tc.tile_pool(name="act", bufs=3) as apool,
            tc.tile_pool(name="psum", bufs=4, space="PSUM") as ppool,
        ):
            wt_sb = cpool.tile([128, 9 * COUT], F32R)
            nc.gpsimd.dma_start(out=wt_sb[:], in_=wt_ext[:])  # casts f32->f32r
            b_sb = cpool.tile([COUT, 1], F32)
            nc.sync.dma_start(out=b_sb[:], in_=b_ext[:])

            for c in range(NCHUNK):
                r0 = c * R
                xt = xpool.tile([128, RIN * WP], F32R)
                xt3 = xt[:].rearrange("p (r c) -> p r c", c=WP)
                if c == 0:
                    # row-slot 0 is the top zero-pad row (covers col 0 too)
                    nc.vector.memset(xt3[:, 0, :].bitcast(F32), 0.0)
                    # zero left-pad column for the remaining slots
                    nc.vector.memset(xt3[:, 1:RIN, 0].bitcast(F32), 0.0)
                    for i in range(PER):
                        nc.gpsimd.dma_start(
                            out=xt3[64 * i : 64 * i + 64, 1:RIN, 1:WP],
                            in_=x_ext[i, :, 0 : 2 * R, :],
                        )
                else:
                    nc.vector.memset(xt3[:, :, 0].bitcast(F32), 0.0)
                    for i in range(PER):
                        nc.gpsimd.dma_start(
                            out=xt3[64 * i : 64 * i + 64, :, 1:WP],
                            in_=x_ext[i, :, 2 * r0 - 1 : 2 * r0 + 2 * R, :],
                        )

                pts = [ppool.tile([128, R * WO], F32, tag="pt", name=f"pt{i}")
                       for i in range(PER)]
                for g in range(R // 4):
                    for t in range(9):
                        ki, kj = divmod(t, 3)
                        for i in range(PER):
                            p0 = 64 * i
                            r_lo = 8 * g + ki
                            rhs = xt3[p0 : p0 + 64, r_lo : r_lo + 7 : 2,
                                      kj : kj + 2 * WO - 1 : 2]
                            lhsT = wt_sb[p0 : p0 + 64, t * COUT : (t + 1) * COUT]
                            nc.tensor.matmul(
                                pts[i][:, g * 512 : (g + 1) * 512],
                                lhsT, rhs,
                                start=(t == 0), stop=(t == 8),
                                tile_position=(p0, 0),
                            )

                for i in range(PER):
                    pt = pts[i]
                    N = R * WO
                    y = apool.tile([128, N], F32)
                    nc.scalar.activation(y[:], pt[:], AFT.Identity,
                                         bias=b_sb[:, 0:1])
                    t_ = apool.tile([128, N], F32)
                    nc.vector.tensor_scalar(t_[:], y[:], -3.0, 3.0,
                                            ALU.max, ALU.min)
                    h = apool.tile([128, N], F32)
                    nc.vector.scalar_tensor_tensor(h[:], t_[:], 3.0, y[:],
                                                   ALU.add, ALU.mult)
                    u = apool.tile([128, N], F32)
                    nc.scalar.activation(u[:], h[:], AFT.Exp, scale=1.0 / 6.0)
                    w2 = apool.tile([128, N], F32)
                    nc.scalar.activation(w2[:], u[:], AFT.Square, bias=1.0)
                    d = apool.tile([128, N], F32)
                    nc.scalar.activation(d[:], w2[:], AFT.Identity, bias=1.0)
                    rcp = apool.tile([128, N], F32)
                    nc.vector.reciprocal_approx_fast(rcp[:], d[:])
                    z = apool.tile([128, N], F32)
                    nc.vector.scalar_tensor_tensor(z[:], w2[:], -1.0, rcp[:],
                                                   ALU.add, ALU.mult)
                    o = apool.tile([128, N], F32)
                    nc.vector.scalar_tensor_tensor(o[:], z[:], 1.0 / 6.0, h[:],
                                                   ALU.mult, ALU.mult)
                    nc.sync.dma_start(
                        out=out_ext[i, :, r0 : r0 + R, :],
                        in_=o[:].rearrange("p (r c) -> p r c", c=WO),
                    )
    nc.compile()
    return nc


def _get_nc():
    if "nc" not in _CACHE:
        _CACHE["nc"] = _build()
    return _CACHE["nc"]


def _prep(x, weight, bias):
    x = np.ascontiguousarray(np.asarray(x, dtype=np.float32))
    w = np.asarray(weight, dtype=np.float32)
    b = np.asarray(bias, dtype=np.float32)
    # [cin, (ki*3+kj)*COUT + cout], duplicated across both partition halves
    wt = np.ascontiguousarray(
        w.transpose(1, 2, 3, 0).reshape(CIN, 9 * COUT)
    )
    wt2 = np.ascontiguousarray(np.concatenate([wt, wt], axis=0))
    b0 = np.ascontiguousarray((b - 0.5).reshape(COUT, 1))
    in_maps = [
        {"x": x[PER * i : PER * (i + 1)], "wt": wt2, "b0": b0}
        for i in range(NCORE)
    ]
    return in_maps


def _run(in_maps, **kw):
    nc = _get_nc()
    return run_bass_kernel_spmd(nc, in_maps, list(range(NCORE)), **kw)


def kernel(x, weight, bias):
    res = _run(_prep(x, weight, bias))
    return np.ascontiguousarray(
        np.concatenate([res.results[i]["out"] for i in range(NCORE)], axis=0)
    )
